# revision 7
# baseline (speedup 1.0000x reference)
# Dual-branch attention (nn_Attention_44702019616803) on 8 TRN2 cores,
# data-parallel over batch (one batch item per core; full inputs in, full
# output gathered on host).
#
# v2 design (vs the bf16 baseline):
#   q/k projections and qk^T scores run in fp8e4m3 with DoubleRow perf
#   mode (2 contraction subtiles per instruction at 0.5 cycles/row): the
#   hd=48 head dim is split 24x2, four channels share a 128-partition
#   group at tile positions {0,32,64,96}; q-bias rides a ones-row in x8
#   so all PSUM->SBUF q/k copies are plain tensor_copy. v and the final
#   projection stay bf16 (fp8 there fails the 2e-2 gate; scores/attn
#   weights in fp8 are precision-free since softmax weight noise
#   averages out in pv).
#   A (unnormalized exp of scores) is stored [n, g, c, mj] (m=4g+mj) so
#   mix-transpose slabs A[:,g,:,:] are contiguous (c,mj)-major; softmax-1
#   denominators come free from Act's accum_out (the 3 m-pad columns
#   contribute exp(0)=1 each, subtracted exactly before the reciprocal);
#   1/Z is folded into the DVE multiply on the mandatory transpose
#   PSUM->SBUF copy (baseline trick, c-major zi replication via one PE
#   transpose of a broadcast zr4).
#   exp2 is split Act/DVE 5:5: the DVE share uses the Schraudolph trick
#   (y = round(mixed*128/ln2 + 16250.5) as int16 = bf16 bits of
#   ~exp(mixed), max 3.3% rel err, harmless after softmax-2).
#   Emission keeps the baseline 5-block software pipeline: block b's
#   scores/exp1 overlap mix(b-1) then pv/wt/proj(b-1); all five v-chunks
#   ride cycle 1's PE slack; the tail runs exp2(4) entirely on Act.
#   All three Z-barrier seams (startup, phase-1->cycle-1, cycle-4->tail)
#   are packed with Z-independent work: first-group k-copies on Act,
#   fine-grained v-pieces, and the carried block-3 pv/wt/proj head.
#   Engine occupancy at 220.2us: Act ~80% (pacer: exp1 107us + accum
#   reads 30us + exp2 share), DVE ~78%, PE ~65%.
import sys
sys.path.insert(0, "/opt/trn_rl_repo")
import numpy as np
import ml_dtypes
import concourse.bacc as bacc
import concourse.bass as bass
import concourse.mybir as mybir
import concourse.tile as tile
from concourse.bass import ds, ts
from concourse.bass_utils import run_bass_kernel_spmd

BF = ml_dtypes.bfloat16
F8 = ml_dtypes.float8_e4m3fn
P = 128
NT = 577
D = 768
H = 16
KO = 7            # 896 = 7*128 contraction tiles for v/proj (768 + bias + pad)
NBLK = [(0, 128), (128, 128), (256, 128), (384, 128), (512, 65)]
MPAD = 580        # m padded to 4*145
NG = 145          # m4 groups
SCALE = 48 ** -0.5
# Schraudolph exp->bf16 bits: y = round(x*A + B), int16 bits viewed as bf16
SCH_A = 128.0 / np.log(2.0)
SCH_B = 127.0 * 128.0 - 5.5
# fraction of exp2 chunks on Act (rest on DVE via Schraudolph)
EXP2_ACT = 5      # out of 10
E1_DVE = set()            # optional exp1 channels on DVE (Schraudolph)

_cache = {}


def _build(nc):
    bf = mybir.dt.bfloat16
    f8 = mybir.dt.float8e4
    f32 = mybir.dt.float32
    i16 = mybir.dt.int16
    EXPF = mybir.ActivationFunctionType.Exp
    DR = mybir.MatmulPerfMode.DoubleRow

    x8_d = nc.dram_tensor("x8T", [P, 4, 2, NT], f8, kind="ExternalInput")
    xf8_d = nc.dram_tensor("xf8T", [P, 4, 2, NT], f8, kind="ExternalInput")
    x_d = nc.dram_tensor("xT", [P, KO, NT], bf, kind="ExternalInput")
    wq_d = nc.dram_tensor("wq8", [P, 4, 2, 1024], f8, kind="ExternalInput")
    wk_d = nc.dram_tensor("wk8", [P, 3, 2, 1024], f8, kind="ExternalInput")
    wv_d = nc.dram_tensor("wvT", [P, KO, 784], bf, kind="ExternalInput")
    pw_d = nc.dram_tensor("pwT", [P, KO, D], bf, kind="ExternalInput")
    wb_d = nc.dram_tensor("wblk", [P, 64], bf, kind="ExternalInput")
    id_d = nc.dram_tensor("idn", [P, P], bf, kind="ExternalInput")
    out_d = nc.dram_tensor("outT", [D, NT], f32, kind="ExternalOutput")

    with tile.TileContext(nc) as tc:
        wp = tc.alloc_tile_pool(name="wp", bufs=1)
        bk = tc.alloc_tile_pool(name="bk", bufs=2)
        ps = tc.alloc_tile_pool(name="ps", bufs=2, space="PSUM")
        ld2 = tc.alloc_tile_pool(name="ld2", bufs=1)   # lives until cycle 1 done
        ld = tc.alloc_tile_pool(name="ld", bufs=1)     # released after phase 1

        # persistent tiles
        wblk = wp.tile([P, 64], bf)
        idn = wp.tile([P, P], bf)
        # q/k fp8 score operands: [p=32*ho+dd (dd<24), cg=4*br+cg0, sub, tok]
        qt8 = wp.tile([P, 8, 2, 640], f8)
        kt8 = wp.tile([P, 8, 2, MPAD], f8)
        v_sb = wp.tile([P, 5, 784], bf)
        pw = wp.tile([P, KO, D], bf)
        wt = wp.tile([P, KO, 584], bf)
        A1 = wp.tile([P, NG, 32, 4], bf)     # [n, g, c, mj]; m=4g+mj
        E2 = wp.tile([P, 16, 640], bf)
        E2_16 = E2.bitcast(i16)

        # load-phase tiles
        wq8 = ld.tile([P, 4, 2, 1024], f8, bufs=1)
        wk8 = ld.tile([P, 3, 2, 1024], f8, bufs=1)
        x8 = ld.tile([P, 4, 2, 584], f8, bufs=1)
        xf8 = ld.tile([P, 4, 2, 584], f8, bufs=1)
        wv = ld2.tile([P, KO, 784], bf, bufs=1)
        x_sb = ld2.tile([P, KO, 584], bf, bufs=1)
        # startup-critical order: first k/q slices need x8 + first wk8/wq8
        # column groups; small idn/wblk early (gate cycle-1 transposes)
        nc.sync.dma_start(x8[:, :, :, :NT], x8_d[:])
        nc.sync.dma_start(wk8[:, :, :, 0:256], wk_d[:, :, :, 0:256])
        nc.sync.dma_start(wq8[:, :, :, 0:256], wq_d[:, :, :, 0:256])
        nc.sync.dma_start(idn[:], id_d[:])
        nc.sync.dma_start(wblk[:], wb_d[:])
        nc.sync.dma_start(wk8[:, :, :, 256:1024], wk_d[:, :, :, 256:1024])
        nc.sync.dma_start(wq8[:, :, :, 256:1024], wq_d[:, :, :, 256:1024])
        nc.sync.dma_start(xf8[:, :, :, :NT], xf8_d[:])
        nc.sync.dma_start(x_sb[:, :, :NT], x_d[:])
        nc.sync.dma_start(wv[:], wv_d[:])
        nc.sync.dma_start(pw[:], pw_d[:])

        nc.gpsimd.memset(qt8[:, :, :, NT:640], 0.0)
        nc.gpsimd.memset(kt8[:, :, :, NT:MPAD], 0.0)
        nc.gpsimd.memset(A1[:, NG - 1, :, 1:4], 0.0)
        nc.gpsimd.memset(E2[:, :, MPAD:640], 0.0)
        nc.gpsimd.memset(wt[:, KO - 1, :], 0.0)
        nc.gpsimd.memset(wt[0:1, KO - 1, :], 1.0)

        As = [A1, None]      # second A allocated after ld release

        # ---- fp8 DoubleRow q/k projection: one 128-col slice ----
        # slice sigma = 2*cg0 + sub of wq8/wk8; dst cg = 4*br + cg0
        def kq_slice(w8, dst, src8, br, cg0, sub, nkp, on_act):
            sig = 2 * cg0 + sub
            cg = 4 * br + cg0
            ppA = ps.tile([P, 512], f32, tag="rp", name="ppA")
            ppB = ps.tile([P, 65], f32, tag="mp", name="ppB")
            for o0, w in ((0, 256), (256, 256)):
                for kp in range(nkp):
                    nc.tensor.matmul(ppA[:, o0:o0 + w], w8[:, kp, :, ts(sig, P)],
                                     src8[:, kp, :, ds(o0, w)],
                                     start=(kp == 0), stop=(kp == nkp - 1),
                                     perf_mode=DR)
            for kp in range(nkp):
                nc.tensor.matmul(ppB[:, 0:65], w8[:, kp, :, ts(sig, P)],
                                 src8[:, kp, :, ds(512, 65)],
                                 start=(kp == 0), stop=(kp == nkp - 1),
                                 perf_mode=DR)
            if on_act:
                nc.scalar.copy(dst[:, cg, sub, 0:512], ppA[:, :])
                nc.scalar.copy(dst[:, cg, sub, 512:NT], ppB[:, :])
            else:
                nc.vector.tensor_copy(dst[:, cg, sub, 0:512], ppA[:, :])
                nc.vector.tensor_copy(dst[:, cg, sub, 512:NT], ppB[:, :])

        def v_piece(mt, half):
            m0, mlen = NBLK[mt]
            o0, w = (0, 512) if half == 0 else (512, 272)
            tag = "rp" if half == 0 else "mp"
            pp = ps.tile([P, w], f32, tag=tag, name="vp")
            for ko in range(KO):
                nc.tensor.matmul(pp[:mlen, :], x_sb[:, ko, ds(m0, mlen)],
                                 wv[:, ko, ds(o0, w)],
                                 start=(ko == 0), stop=(ko == KO - 1))
            nc.vector.tensor_copy(v_sb[:mlen, mt, o0:o0 + w], pp[:mlen, :])

        def v_chunk(mt):
            v_piece(mt, 0)
            v_piece(mt, 1)

        # ---- per-block score channel: fp8 DR, 24 contraction rows x2 ----
        def score_c(bi, c, zt_box):
            n0, nlen = NBLK[bi]
            A = As[bi % 2]
            br, h = c // 16, c % 16
            cg = 4 * br + (h // 4)
            poff = 32 * (h % 4)
            sp = ps.tile([P, MPAD], f32, tag="sp", name="sp")
            q_sl = qt8[poff:poff + 24, cg, :, ds(n0, nlen)]
            for m0, w in ((0, 256), (256, 256), (512, 68)):
                nc.tensor.matmul(sp[:nlen, m0:m0 + w], q_sl,
                                 kt8[poff:poff + 24, cg, :, ds(m0, w)],
                                 start=True, stop=True, perf_mode=DR,
                                 tile_position=(poff, 0), skip_group_check=True)
            # pad m (577-579) scores are 0 (kt8 pad zeroed) -> exp=1; the +3
            # on Z is subtracted exactly in zi_chain.
            if bi == 0 and c in E1_DVE:
                A16 = A.bitcast(mybir.dt.int16)
                nc.vector.tensor_scalar(A16[:, :, c, :], sp[:, :MPAD],
                                        SCH_A * SCALE, SCH_B,
                                        mybir.AluOpType.mult,
                                        mybir.AluOpType.add)
                nc.vector.tensor_reduce(zt_box[0][:, c:c + 1], A[:, :, c, :],
                                        mybir.AxisListType.XY,
                                        mybir.AluOpType.add)
            else:
                nc.scalar.activation(A[:, :, c, :], sp[:, :MPAD], EXPF,
                                     scale=SCALE,
                                     accum_out=zt_box[0][:, c:c + 1])



        # ---- pv stage (baseline structure; reads E2/v_sb) ----
        def make_pv(w_acc):
            e2tgs = {}
            boxes = {}

            def tpiece(og2, mt, on_act=False, tag="rp"):
                # two og (8 heads) per transpose+copy chunk
                tp = ps.tile([P, 8, P], bf, tag=tag, name="tp")
                for oj in range(8):
                    o = 8 * og2 + oj
                    nc.tensor.transpose(tp[:, oj, :], E2[:, o, ds(P * mt, P)], idn[:])
                for h in range(2):
                    og = 2 * og2 + h
                    if mt == 0:
                        e2tgs[og] = bk.tile([P, 4, 5, P], bf, tag="e2tg", bufs=4,
                                            name="e2tg")
                if on_act:
                    nc.scalar.copy(e2tgs[2 * og2][:, :, mt, :], tp[:, 0:4, :])
                    nc.scalar.copy(e2tgs[2 * og2 + 1][:, :, mt, :], tp[:, 4:8, :])
                else:
                    nc.vector.tensor_copy(e2tgs[2 * og2][:, :, mt, :], tp[:, 0:4, :])
                    nc.vector.tensor_copy(e2tgs[2 * og2 + 1][:, :, mt, :],
                                          tp[:, 4:8, :])

            def mpiece(og, oj):
                if og not in boxes:
                    boxes[og] = ps.tile([P, 4, 49], f32, tag="mp", name="pv4")
                pv4 = boxes[og]
                o = 4 * og + oj
                for mt, (m0, mlen) in enumerate(NBLK):
                    nc.tensor.matmul(pv4[:, oj, :], e2tgs[og][:mlen, oj, mt, :],
                                     v_sb[:mlen, mt, ds(49 * o, 49)],
                                     start=(mt == 0), stop=(mt == 4))

            def norm(og):
                pv4 = boxes[og]
                zr4 = bk.tile([P, 4], f32, tag="zr4", bufs=2, name="zr4")
                nc.vector.reciprocal(zr4[:], pv4[:, :, 48])
                nc.vector.tensor_mul(w_acc[:, ds(4 * og, 4), :], pv4[:, :, :48],
                                     zr4.unsqueeze(2).broadcast_to([P, 4, 48]))

            return tpiece, mpiece, norm

        def pv_pieces(bi, w_acc):
            tpiece, mpiece, norm = make_pv(w_acc)
            tps = [lambda og2=og2, mt=mt: tpiece(og2, mt)
                   for og2 in range(2) for mt in range(5)]
            def bundle(og):
                for oj in range(4):
                    mpiece(og, oj)
                norm(og)
            ms = [lambda og=og: bundle(og) for og in range(4)]
            return tps, ms

        def wt_one(bi, w_acc, j, on_act=False):
            n0, nlen = NBLK[bi]
            wa = w_acc.rearrange("p o d -> p (o d)")
            wpp = ps.tile([P, P], bf, tag="rp", name="wpp")
            nc.tensor.transpose(wpp[:, :nlen], wa[:nlen, ds(P * j, P)],
                                idn[:nlen, :nlen])
            if on_act:
                nc.scalar.copy(wt[:, j, n0:n0 + nlen], wpp[:, :nlen])
            else:
                nc.vector.tensor_copy(wt[:, j, n0:n0 + nlen], wpp[:, :nlen])

        def wt_pieces(bi, w_acc, on_act=False):
            for j in range(0, 6, 2):
                yield lambda j=j: (wt_one(bi, w_acc, j, on_act),
                                   wt_one(bi, w_acc, j + 1, on_act))

        out_v = out_d.rearrange("(dt p) m -> p dt m", p=P)

        def proj_pieces(bi, on_act=False):
            n0, nlen = NBLK[bi]
            obx = [None]

            def one(dt):
                fp = ps.tile([P, P], f32, tag="mp", name="fpj")
                for ko in range(KO):
                    nc.tensor.matmul(fp[:, :nlen], pw[:, ko, ts(dt, P)],
                                     wt[:, ko, ds(n0, nlen)],
                                     start=(ko == 0), stop=(ko == KO - 1))
                if obx[0] is None:
                    obx[0] = bk.tile([P, 6, P], f32, tag="ob", bufs=1, name="ob")
                ob = obx[0]
                act = on_act if not on_act else (dt % 2 == 0)
                if act:
                    nc.scalar.copy(ob[:, dt, :nlen], fp[:, :nlen])
                else:
                    nc.vector.tensor_copy(ob[:, dt, :nlen], fp[:, :nlen])
                if dt == 5:
                    nc.sync.dma_start(out_v[:, :, n0:n0 + nlen], ob[:, :, :nlen])
            for dt in range(6):
                yield lambda dt=dt: one(dt)

        # ---- mix pieces: transpose A [n,(c,mj)] slabs, fused 1/Z multiply,
        # block-diag conv mm, exp2 on Act or DVE (Schraudolph) ----
        E2v = E2[:, :, :MPAD].rearrange("p o (g mj) -> p g mj o", mj=4)
        E2v16 = E2_16[:, :, :MPAD].rearrange("p o (g mj) -> p g mj o", mj=4)

        def mix_pieces(bi, zi_box, act_all=False, act_n=None):
            A = As[bi % 2]
            st = {"prev": None, "ci": 0, "an": EXP2_ACT if act_n is None else act_n}

            def emit_mm(rs, gq, ng):
                mp = ps.tile([P, 8, 64], f32, tag="mp", name="mp")  # 8-slab
                for gi in range(ng):
                    nc.tensor.matmul(mp[:, gi, :], rs[:, gi, :], wblk[:],
                                     start=True, stop=True)
                mpv = mp.rearrange("p g (mj o) -> p g mj o", o=16)
                ci = st["ci"]
                st["ci"] += 1
                if act_all or ci % 10 < st["an"]:
                    nc.scalar.activation(E2v[:, ds(gq, ng), :, :],
                                         mpv[:, :ng, :, :], EXPF)
                else:
                    nc.vector.tensor_scalar(E2v16[:, ds(gq, ng), :, :],
                                            mpv[:, :ng, :, :], SCH_A, SCH_B,
                                            mybir.AluOpType.mult,
                                            mybir.AluOpType.add)

            def chunk(gq, ng):
                rp = ps.tile([P, 8, P], bf, tag="rp", name="rp")
                for gi in range(ng):
                    nc.tensor.transpose(rp[:, gi, :], A[:, gq + gi, :, :], idn[:])
                rs = bk.tile([P, 8, P], bf, tag="rs", bufs=2, name="rs")
                nc.vector.tensor_mul(rs[:, :ng, :], rp[:, :ng, :],
                                     zi_box[0].unsqueeze(1).broadcast_to([P, ng, P]))
                if st["prev"] is not None:
                    emit_mm(*st["prev"])
                st["prev"] = (rs, gq, ng)

            for gq in range(0, NG, 8):
                yield lambda gq=gq: chunk(gq, min(8, NG - gq))
            yield lambda: emit_mm(*st["prev"])

        # 1/Z replicated to [(c,mj), n] layout: zr4 bcast + one transpose
        def zi_chain(zt_k, zi_box):
            ztf = wp.tile([P, 32], f32, tag="ztf", bufs=2, name="ztf")
            nc.vector.tensor_scalar_add(ztf[:], zt_k[:], -3.0)
            zr_k = wp.tile([P, 32], bf, tag="zr", bufs=2, name="zr")
            with nc.allow_low_precision(reason="bf16 recip of softmax denom"):
                nc.vector.reciprocal(zr_k[:], ztf[:])
            zr4 = wp.tile([P, 32, 4], bf, tag="zr4r", bufs=2, name="zr4r")
            nc.vector.tensor_copy(zr4[:], zr_k.unsqueeze(2).broadcast_to([P, 32, 4]))
            zpT = ps.tile([P, P], bf, tag="rp", name="zpT")
            nc.tensor.transpose(zpT[:, :], zr4.rearrange("p c m -> p (c m)"), idn[:])
            zi_rep = bk.tile([P, P], bf, tag="zi", bufs=2, name="zi")
            nc.vector.tensor_copy(zi_rep[:], zpT[:])
            zi_box[0] = zi_rep

        # ================= emission =================
        # phase 1: fp8 q/k projections with block-0 scores+exp1 interleaved.
        zt_ks = [wp.tile([P, 32], f32, tag="zt", bufs=2, name="zt0")]
        nslc = 0
        for br, src8 in ((0, x8), (1, xf8)):
            for cg0 in range(4):
                for sub in range(2):
                    kq_slice(wk8, kt8, src8, br, cg0, sub, 3, nslc < 2)
                    nslc += 1
                for sub in range(2):
                    kq_slice(wq8, qt8, src8, br, cg0, sub, 4, False)
                    nslc += 1
                for ho in range(4):
                    c = 16 * br + 4 * cg0 + ho
                    score_c(0, c, [zt_ks[0]])


        # wq8/wk8/x8/xf8 dead from here; reuse their space for the second A
        ld.release()
        db = tc.alloc_tile_pool(name="db", bufs=1)
        A2 = db.tile([P, NG, 32, 4], bf)
        nc.vector.memset(A2[:, NG - 1, :, 1:4], 0.0)
        As[1] = A2

        # cycles: block bi scores/exp1 overlap mix(bi-1) then pv/wt/proj(bi-1).
        # v chunks 2-4 ride in cycle 1's early window.
        carry = []
        for bi in range(1, 5):
            zt_k = wp.tile([P, 32], f32, tag="zt", bufs=2, name="zt")
            zt_ks.append(zt_k)
            w_acc = bk.tile([P, 16, 48], bf, tag="wacc", bufs=2, name="wacc")
            zi_box = [None]
            zi_chain(zt_ks[bi - 1], zi_box)
            early = list(mix_pieces(bi - 1, zi_box))           # 20 pieces
            if bi == 1:
                # v fills the Z(0)-barrier gap: PE matmuls + DVE copies run
                # while Act drains the last exp1(0) accums; fine-grained
                # pieces so scores(1) are never delayed long on PE
                vs = [lambda mt=mt, h=h: v_piece(mt, h)
                      for mt in range(5) for h in range(2)]
                mixed = vs[:2]
                vi = 2
                for i, p in enumerate(early):
                    mixed.append(p)
                    if i % 2 == 0 and vi < len(vs):
                        mixed.append(vs[vi])
                        vi += 1
                mixed.extend(vs[vi:])
                early = mixed
            if carry:
                early = [p for pair in zip(early[:len(carry)], carry)
                         for p in pair] + early[len(carry):]
            late, pv_ms = pv_pieces(bi - 1, w_acc)             # 20 + 16 pieces
            carry = pv_ms + list(wt_pieces(bi - 1, w_acc)) + \
                list(proj_pieces(bi - 1))
            NE = len(early)
            NL = len(late)
            edone = ldone = 0
            zt_box = [zt_k]
            hd = 0   # exp1 head start disabled
            for c in range(32):
                score_c(bi, c, zt_box)
                if c < hd:
                    continue
                if c < 19:
                    want = (c + 1 - hd) * NE // (19 - hd)
                    while edone < want:
                        early[edone]()
                        edone += 1
                else:
                    want = (c - 18) * NL // 13
                    while ldone < want:
                        ldone += 1
                        late[ldone - 1]()
            while edone < NE:
                early[edone]()
                edone += 1
            while ldone < NL:
                late[ldone]()
                ldone += 1

        # tail: run mix(4), then flush carried pieces and pv/wt/proj(4)
        zi_box = [None]
        zi_chain(zt_ks[4], zi_box)
        tail_carry = list(carry)
        w_acc4 = bk.tile([P, 16, 48], bf, tag="wacc", bufs=2, name="wacc4")
        tpiece, mpiece, norm = make_pv(w_acc4)
        # pv(3) bundles + first carried pieces fill the Z(4)-barrier gap
        # (mix(4) rs-chain stalls on the last exp1(4) accum)
        for b in tail_carry[:4]:
            b()
        rest = tail_carry[4:]
        head, rest = rest[:6], rest[6:]
        for b in head:
            b()
        for piece in mix_pieces(4, zi_box, act_all=True):
            piece()
        k = 0
        for og2 in range(2):
            for mt in range(5):
                i = og2 * 5 + mt
                tpiece(og2, mt, on_act=i % 2 == 0,
                       tag="rp" if i % 2 == 0 else "sp")
                if k < len(rest):
                    rest[k]()
                    k += 1
                if k < len(rest):
                    rest[k]()
                    k += 1
        while k < len(rest):
            rest[k]()
            k += 1
        wt_after = {0: [0], 1: [1], 2: [2, 3], 3: [4, 5]}
        for og in range(4):
            for oj in range(4):
                mpiece(og, oj)
            norm(og)
            for j in wt_after[og]:
                wt_one(4, w_acc4, j, on_act=(j % 2 == 0))
        for i, piece in enumerate(proj_pieces(4, on_act=True)):
            piece()

        db.release()
        ld2.release()
        ps.release()
        bk.release()
        wp.release()
    nc.finalize()
    return nc


def _shuf(a, npair, s=2):
    # [rows, cols] -> [128, npair, s, cols], row = (kp*s + sub)*128 + p
    rows = npair * s * P
    out = np.zeros((rows, a.shape[1]), a.dtype)
    out[:a.shape[0]] = a
    return out.reshape(npair, s, P, a.shape[1]).transpose(2, 0, 1, 3).copy()


def _shufko(a, ko):
    out = np.zeros((ko * P, a.shape[1]), a.dtype)
    out[:a.shape[0]] = a
    return out.reshape(ko, P, a.shape[1]).transpose(1, 0, 2).copy()


def _prep_weights(qkv_w, qkv_b, conv_w, proj_w, proj_b):
    f = np.float32
    qkv_w, qkv_b = qkv_w.astype(f), qkv_b.astype(f)
    # wq8/wk8 [768, 1024]: slice sig = 2*cg0+sub (128 cols each);
    # col within slice = 32*ho + dd, head h = 4*cg0 + ho, d = 24*sub + dd
    wq = np.zeros((1024, 1024), f)
    wk = np.zeros((768, 1024), f)
    for h in range(H):
        cg0, ho = h // 4, h % 4
        for sub in range(2):
            sig = 2 * cg0 + sub
            cols = slice(128 * sig + 32 * ho, 128 * sig + 32 * ho + 24)
            rows = slice(48 * h + 24 * sub, 48 * h + 24 * sub + 24)
            wq[:768, cols] = qkv_w[rows, :].T
            wk[:, cols] = qkv_w[768 + 48 * h + 24 * sub:
                                768 + 48 * h + 24 * sub + 24, :].T
            wq[768, cols] = qkv_b[rows]
    wv = np.zeros((896, 784), f)
    for h in range(H):
        wv[:768, 49 * h:49 * h + 48] = qkv_w[1536 + 48 * h:1536 + 48 * h + 48, :].T
        wv[768, 49 * h:49 * h + 48] = qkv_b[1536 + 48 * h:1536 + 48 * h + 48]
        wv[768, 49 * h + 48] = 1.0
    pwm = np.zeros((896, D), f)
    pwm[:768] = proj_w.astype(f).T
    pwm[768] = proj_b.astype(f)
    # wblk rows (c, mj) c-major: row 4c+mj; cols (mj', o): col 16*mj'+o
    wblk = np.zeros((128, 64), f)
    for c in range(32):
        for mj in range(4):
            wblk[4 * c + mj, 16 * mj:16 * mj + 16] = conv_w[:, c].astype(f)
    idn = np.eye(128, dtype=f)
    return {"wq8": _shuf(wq.astype(F8), 4), "wk8": _shuf(wk.astype(F8), 3),
            "wvT": _shufko(wv.astype(BF), KO), "pwT": _shufko(pwm.astype(BF), KO),
            "wblk": wblk.astype(BF), "idn": idn.astype(BF)}


def kernel(x, x_freq, qkv_w, qkv_b, conv_w, conv_b, proj_w, proj_b, _profile=False):
    # conv_b is constant along the softmax axis -> cancels in softmax; unused.
    if "nc" not in _cache:
        _cache["nc"] = _build(bacc.Bacc())
    nc = _cache["nc"]
    wmap = _prep_weights(np.asarray(qkv_w), np.asarray(qkv_b), np.asarray(conv_w),
                         np.asarray(proj_w), np.asarray(proj_b))
    B = x.shape[0]
    in_maps = []
    for b in range(B):
        xb = np.asarray(x[b], np.float32).T
        xfb = np.asarray(x_freq[b], np.float32).T
        xT = np.zeros((896, NT), np.float32)
        xT[:768] = xb
        xT[768] = 1.0
        x8b = np.zeros((1024, NT), np.float32)
        x8b[:768] = xb
        x8b[768] = 1.0
        xf8b = np.zeros((1024, NT), np.float32)
        xf8b[:768] = xfb
        xf8b[768] = 1.0
        in_maps.append({"x8T": _shuf(x8b.astype(F8), 4),
                        "xf8T": _shuf(xf8b.astype(F8), 4),
                        "xT": _shufko(xT.astype(BF), KO), **wmap})
    res = run_bass_kernel_spmd(nc, in_maps, core_ids=list(range(B)), trace=_profile)
    out = np.stack([res.results[b]["outT"].T for b in range(B)], axis=0)
    if _profile:
        return out.astype(np.float32), res
    return out.astype(np.float32)
